# revision 13
# baseline (speedup 1.0000x reference)
"""Trainium2 Bass kernel for nn_Attention_57346403336437.

Math (per sample n):
    h1[t, :] = tanh(x[n,t,:] @ W1[:256] + y[n,:] @ W1[256:] + b1)      (T, 32)
    h2[t]    = relu(h1[t, :] @ W2 + b2)                                 (T,)
    a[t]     = exp(h2[t]) / (sum_t exp(h2[t]) + 1e-7)

Sharding: data-parallel over batch N=256 across 8 cores (32 samples each).
Weights replicated. Softmax-like normalization over T is core-local.

Device layout (per core):
  - x arrives host-transposed as xt (32, 256, 512) = (n, d, t) so the
    contraction dim d sits on SBUF partitions with fully contiguous DMA;
    loads batched 4 samples (2 MiB) per dma_start for near-peak HBM BW.
  - q matmul per sample: psum[32h, 512t] += W1x_chunk.T @ xt_chunk, fp32r.
  - h2 accumulated into one [32, 512] psum via per-sample shifted
    block-diagonal W2 columns so every matmul writes at partition base 0
    (fp32r rejects col-offset PSUM writes).
  - epilogue: exp (with b2 bias), clamp-at-1 (== exp o relu), row-sum,
    reciprocal, scale, store.
"""

import numpy as np

N, T, D, HID = 256, 512, 256, 32
NCORES = 8
NS = N // NCORES          # samples per core
GS = 4                    # samples per DMA batch
NG = NS // GS             # batches per core
EPS = 1e-7

# matmul input dtype: "f32r" (fast, reduced precision) or "f32" (exact, 4x slower)
MM_MODE = "f32r"
W2COLS = 2 * NS - 1


def build_bass():
    import concourse.bacc as bacc
    import concourse.tile as tile
    from concourse import mybir

    f32 = mybir.dt.float32
    fmm = mybir.dt.float32r if MM_MODE == "f32r" else f32
    nc = bacc.Bacc()

    xt = nc.declare_dram_parameter("xt", [NS, D, T], fmm, isOutput=False)
    ytp = nc.declare_dram_parameter("ytp", [128, 2, NS], fmm, isOutput=False)
    w1p = nc.declare_dram_parameter("w1p", [128, 4, HID], fmm, isOutput=False)
    b1r = nc.declare_dram_parameter("b1r", [HID, 1], f32, isOutput=False)
    w2s = nc.declare_dram_parameter("w2s", [HID, W2COLS], f32, isOutput=False)
    b2r = nc.declare_dram_parameter("b2r", [NS, 1], f32, isOutput=False)
    out = nc.declare_dram_parameter("out", [NS, T], f32, isOutput=True)

    with tile.TileContext(nc) as tc:
        with (
            tc.tile_pool(name="singles", bufs=1) as singles,
            tc.tile_pool(name="xpool", bufs=3) as xpool,
            tc.tile_pool(name="hpool", bufs=6) as hpool,
            tc.tile_pool(name="opool", bufs=1) as opool,
            tc.tile_pool(name="psq", bufs=6, space="PSUM") as psq,
            tc.tile_pool(name="psc", bufs=1, space="PSUM") as psc,
            tc.tile_pool(name="psh", bufs=1, space="PSUM") as psh,
        ):
            # ---- one-time constants ----
            w1sb = singles.tile([128, 4, HID], fmm)     # [p, a, h]; d = a*128+p
            nc.sync.dma_start(out=w1sb, in_=w1p[:])
            ytsb = singles.tile([128, 2, NS], fmm)      # [p, a, n]
            nc.sync.dma_start(out=ytsb, in_=ytp[:])
            b1sb = singles.tile([HID, 1], f32)
            nc.sync.dma_start(out=b1sb, in_=b1r[:])
            w2sb = singles.tile([HID, W2COLS], f32)     # w2s[h, NS-1] = W2[h]
            nc.sync.dma_start(out=w2sb, in_=w2s[:])
            b2sb = singles.tile([NS, 1], f32)
            nc.sync.dma_start(out=b2sb, in_=b2r[:])

            # ---- per-sample tanh bias: cbb[h, n] = (y[n] @ W1y)[h] + b1[h] ----
            cps = psc.tile([HID, NS], f32)
            for a in range(2):
                nc.tensor.matmul(
                    out=cps,
                    lhsT=w1sb[:, 2 + a, :],
                    rhs=ytsb[:, a, :],
                    start=(a == 0),
                    stop=(a == 1),
                )
            cbb = singles.tile([HID, NS], f32)
            nc.vector.tensor_scalar_add(out=cbb, in0=cps, scalar1=b1sb)

            # ---- main loop ----
            h2ps = psh.tile([NS, T], f32)               # accumulated across samples
            for i in range(NG):
                xg = xpool.tile([128, GS, 2, T], fmm)   # [p, j, a, t]
                nc.sync.dma_start(
                    out=xg,
                    in_=xt[GS * i:GS * i + GS].rearrange("n (a p) t -> p n a t", p=128),
                )
                for j in range(GS):
                    n = GS * i + j
                    qps = psq.tile([HID, T], f32)
                    for a in range(2):
                        nc.tensor.matmul(
                            out=qps,
                            lhsT=w1sb[:, a, :],
                            rhs=xg[:, j, a, :],
                            start=(a == 0),
                            stop=(a == 1),
                        )
                    h1 = hpool.tile([HID, T], f32)
                    nc.scalar.activation(
                        out=h1, in_=qps,
                        func=mybir.ActivationFunctionType.Tanh,
                        bias=cbb[:, n:n + 1], scale=1.0,
                    )
                    # shifted block-diag W2 column: only row n of h2ps gets
                    # this sample's h2
                    nc.tensor.matmul(
                        out=h2ps,
                        lhsT=w2sb[:, NS - 1 - n:2 * NS - 1 - n],
                        rhs=h1,
                        start=(n == 0),
                        stop=(n == NS - 1),
                    )

            # ---- epilogue: a = max(exp(h2 + b2), 1) / (sum_t + eps) ----
            e = opool.tile([NS, T], f32)
            nc.scalar.activation(
                out=e, in_=h2ps,
                func=mybir.ActivationFunctionType.Exp,
                bias=b2sb, scale=1.0,
            )
            ec = opool.tile([NS, T], f32)
            sums = opool.tile([NS, 1], f32)
            nc.vector.tensor_scalar(
                out=ec, in0=e, scalar1=1.0, scalar2=None,
                op0=mybir.AluOpType.max, op1=mybir.AluOpType.add,
                accum_out=sums,
            )
            seps = opool.tile([NS, 1], f32)
            nc.vector.tensor_scalar_add(out=seps, in0=sums, scalar1=EPS)
            rec = opool.tile([NS, 1], f32)
            nc.vector.reciprocal(out=rec, in_=seps)
            aout = opool.tile([NS, T], f32)
            nc.vector.tensor_scalar_mul(out=aout, in0=ec, scalar1=rec)
            nc.sync.dma_start(out=out[:], in_=aout)

    if not nc.is_finalized():
        nc.finalize()
    return nc


def make_in_maps(x, y, W1, b1, W2, b2):
    x = np.asarray(x, dtype=np.float32)
    y = np.asarray(y, dtype=np.float32)
    W1 = np.asarray(W1, dtype=np.float32)
    b1 = np.asarray(b1, dtype=np.float32).reshape(HID)
    W2 = np.asarray(W2, dtype=np.float32).reshape(HID, 1)
    b2 = np.asarray(b2, dtype=np.float32).reshape(1)

    w1p = np.ascontiguousarray(W1.reshape(4, 128, HID).transpose(1, 0, 2))
    b1r = np.ascontiguousarray(b1.reshape(HID, 1))
    w2s = np.zeros((HID, W2COLS), np.float32)
    w2s[:, NS - 1] = W2[:, 0]
    b2r = np.full((NS, 1), b2[0], np.float32)

    in_maps = []
    for c in range(NCORES):
        sl = slice(c * NS, (c + 1) * NS)
        xt = np.ascontiguousarray(x[sl].transpose(0, 2, 1))          # (NS, D, T)
        yc = y[sl]                                                    # (NS, D)
        ytp = np.ascontiguousarray(yc.T.reshape(2, 128, NS).transpose(1, 0, 2))
        in_maps.append({
            "xt": xt, "ytp": ytp, "w1p": w1p, "b1r": b1r,
            "w2s": w2s, "b2r": b2r,
        })
    return in_maps


def run(x, y, W1, b1, W2, b2, trace=False, **run_kwargs):
    from concourse import bass_utils
    nc = build_bass()
    in_maps = make_in_maps(x, y, W1, b1, W2, b2)
    res = bass_utils.run_bass_kernel_spmd(
        nc, in_maps, core_ids=list(range(NCORES)), trace=trace, **run_kwargs,
    )
    shards = [np.asarray(res.results[c]["out"]) for c in range(NCORES)]
    full = np.concatenate(shards, axis=0).reshape(N, T, 1).astype(np.float32)
    return full, res


def kernel(x, y, x_length, W1, b1, W2, b2):
    full, _ = run(x, y, W1, b1, W2, b2, trace=False)
    return full


# revision 14
# speedup vs baseline: 2.3160x; 2.3160x over previous
"""Trainium2 Bass kernel for nn_Attention_57346403336437.

Math (per sample n):
    h1[t, :] = tanh(x[n,t,:] @ W1[:256] + y[n,:] @ W1[256:] + b1)      (T, 32)
    h2[t]    = relu(h1[t, :] @ W2 + b2)                                 (T,)
    a[t]     = exp(h2[t]) / (sum_t exp(h2[t]) + 1e-7)

Sharding: data-parallel over batch N=256 across 8 cores (32 samples each).
Weights replicated. Softmax-like normalization over T is core-local.

Device layout (per core):
  - x arrives host-pretiled as xh (NG, 128, GS, 2, T): partition dim =
    d%128, so every group load is one fully-linear dma_start (16KB
    contiguous per partition) -> near-peak HBM bandwidth.
  - bf16 mode (default): 4 samples packed per PSUM tile [128(4n x 32h), T];
    q matmuls col-tiled at tile_position (0, 32j); one full-width tanh per
    group; h2 via shifted block-diagonal W2 accumulated into one [32, T]
    PSUM tile at partition base 0.
  - f32r fallback: fp32r forbids col-offset PSUM writes, so samples are
    processed one [32h, T] PSUM tile at a time (all matmuls base 0).
  - epilogue: exp (with b2 bias), clamp-at-1 (== exp o relu), row-sum,
    reciprocal, scale, store.
"""

import numpy as np

N, T, D, HID = 256, 512, 256, 32
NCORES = 8
NS = N // NCORES          # samples per core
GS = 4                    # samples per group / DMA batch
NG = NS // GS             # groups per core
EPS = 1e-7

# "bf16": fast (half DMA traffic, 1 cyc/row matmuls), ~5e-3 scale-rel err
# "f32r": precise (~2e-4), full f32 DMA traffic, ~2 cyc/row matmuls
MM_MODE = "bf16"


def _np_mm_dtype():
    if MM_MODE == "bf16":
        import ml_dtypes
        return ml_dtypes.bfloat16
    return np.float32


def build_bass():
    import concourse.bacc as bacc
    import concourse.tile as tile
    from concourse import mybir

    f32 = mybir.dt.float32
    fmm = {"bf16": mybir.dt.bfloat16,
           "f32r": mybir.dt.float32r,
           "f32": f32}[MM_MODE]
    packed = MM_MODE == "bf16"
    nc = bacc.Bacc()

    xh = nc.declare_dram_parameter("xh", [NG, 128, GS, 2, T], fmm, isOutput=False)
    ytp = nc.declare_dram_parameter("ytp", [128, 2, NS], fmm, isOutput=False)
    w1p = nc.declare_dram_parameter("w1p", [128, 4, HID], fmm, isOutput=False)
    out = nc.declare_dram_parameter("out", [NS, T], f32, isOutput=True)
    if packed:
        b1r = nc.declare_dram_parameter("b1r", [128, 1], f32, isOutput=False)
        w2s = nc.declare_dram_parameter("w2s", [128, GS * (NG - 1) + HID], fmm, isOutput=False)
    else:
        b1r = nc.declare_dram_parameter("b1r", [HID, 1], f32, isOutput=False)
        w2s = nc.declare_dram_parameter("w2s", [HID, 2 * NS - 1], fmm, isOutput=False)
    b2r = nc.declare_dram_parameter("b2r", [NS, 1], f32, isOutput=False)

    with tile.TileContext(nc) as tc:
        with (
            tc.tile_pool(name="singles", bufs=1) as singles,
            tc.tile_pool(name="xpool", bufs=3) as xpool,
            tc.tile_pool(name="hpool", bufs=6) as hpool,
            tc.tile_pool(name="opool", bufs=1) as opool,
            tc.tile_pool(name="psq", bufs=6, space="PSUM") as psq,
            tc.tile_pool(name="psc", bufs=1, space="PSUM") as psc,
            tc.tile_pool(name="psh", bufs=1, space="PSUM") as psh,
        ):
            # ---- one-time constants ----
            w1sb = singles.tile([128, 4, HID], fmm)     # [p, a, h]; d = a*128+p
            nc.sync.dma_start(out=w1sb, in_=w1p[:])
            ytsb = singles.tile([128, 2, NS], fmm)      # [p, a, n]
            nc.sync.dma_start(out=ytsb, in_=ytp[:])
            b1sb = singles.tile(list(b1r.shape), f32)
            nc.sync.dma_start(out=b1sb, in_=b1r[:])
            w2sb = singles.tile(list(w2s.shape), fmm)
            nc.sync.dma_start(out=w2sb, in_=w2s[:])
            b2sb = singles.tile([NS, 1], f32)
            nc.sync.dma_start(out=b2sb, in_=b2r[:])

            Tanh = mybir.ActivationFunctionType.Tanh
            h2ps = psh.tile([NS, T], f32)               # accumulated over samples

            if packed:
                # cb[(j,h), i] = (y[4i+j] @ W1y)[h] + b1[h]
                cps = psc.tile([128, NG], f32)
                for j in range(GS):
                    for a in range(2):
                        nc.tensor.matmul(
                            out=cps[32 * j:32 * j + 32, :],
                            lhsT=w1sb[:, 2 + a, :],
                            rhs=ytsb[:, a, :].rearrange("p (i g) -> p g i", g=GS)[:, j, :],
                            start=(a == 0), stop=(a == 1),
                            tile_position=(0, 32 * j),
                        )
                cb = singles.tile([128, NG], f32)
                nc.vector.tensor_scalar_add(out=cb, in0=cps, scalar1=b1sb)

                for i in range(NG):
                    xg = xpool.tile([128, GS, 2, T], fmm)
                    nc.sync.dma_start(out=xg, in_=xh[i])
                    qps = psq.tile([128, T], f32)
                    for j in range(GS):
                        for a in range(2):
                            nc.tensor.matmul(
                                out=qps[32 * j:32 * j + 32, :],
                                lhsT=w1sb[:, a, :],
                                rhs=xg[:, j, a, :],
                                start=(a == 0), stop=(a == 1),
                                tile_position=(0, 32 * j),
                            )
                    h1 = hpool.tile([128, T], fmm)
                    nc.scalar.activation(out=h1, in_=qps, func=Tanh,
                                         bias=cb[:, i:i + 1], scale=1.0)
                    # shifted block-diag W2: rows 4i..4i+3 of h2ps get this
                    # group's scores, zero elsewhere; accumulate over groups
                    nc.tensor.matmul(
                        out=h2ps,
                        lhsT=w2sb[:, GS * (NG - 1 - i):GS * (NG - 1 - i) + HID],
                        rhs=h1,
                        start=(i == 0), stop=(i == NG - 1),
                    )
            else:
                # cbb[h, n] = (y[n] @ W1y)[h] + b1[h]
                cps = psc.tile([HID, NS], f32)
                for a in range(2):
                    nc.tensor.matmul(out=cps, lhsT=w1sb[:, 2 + a, :],
                                     rhs=ytsb[:, a, :],
                                     start=(a == 0), stop=(a == 1))
                cbb = singles.tile([HID, NS], f32)
                nc.vector.tensor_scalar_add(out=cbb, in0=cps, scalar1=b1sb)

                for i in range(NG):
                    xg = xpool.tile([128, GS, 2, T], fmm)
                    nc.sync.dma_start(out=xg, in_=xh[i])
                    for j in range(GS):
                        n = GS * i + j
                        qps = psq.tile([HID, T], f32)
                        for a in range(2):
                            nc.tensor.matmul(out=qps, lhsT=w1sb[:, a, :],
                                             rhs=xg[:, j, a, :],
                                             start=(a == 0), stop=(a == 1))
                        h1 = hpool.tile([HID, T], fmm if MM_MODE == "bf16" else f32)
                        nc.scalar.activation(out=h1, in_=qps, func=Tanh,
                                             bias=cbb[:, n:n + 1], scale=1.0)
                        nc.tensor.matmul(
                            out=h2ps,
                            lhsT=w2sb[:, NS - 1 - n:2 * NS - 1 - n],
                            rhs=h1,
                            start=(n == 0), stop=(n == NS - 1),
                        )

            # ---- epilogue: a = max(exp(h2 + b2), 1) / (sum_t + eps) ----
            e = opool.tile([NS, T], f32)
            nc.scalar.activation(out=e, in_=h2ps,
                                 func=mybir.ActivationFunctionType.Exp,
                                 bias=b2sb, scale=1.0)
            ec = opool.tile([NS, T], f32)
            sums = opool.tile([NS, 1], f32)
            nc.vector.tensor_scalar(
                out=ec, in0=e, scalar1=1.0, scalar2=None,
                op0=mybir.AluOpType.max, op1=mybir.AluOpType.add,
                accum_out=sums,
            )
            seps = opool.tile([NS, 1], f32)
            nc.vector.tensor_scalar_add(out=seps, in0=sums, scalar1=EPS)
            rec = opool.tile([NS, 1], f32)
            nc.vector.reciprocal(out=rec, in_=seps)
            aout = opool.tile([NS, T], f32)
            nc.vector.tensor_scalar_mul(out=aout, in0=ec, scalar1=rec)
            nc.sync.dma_start(out=out[:], in_=aout)

    if not nc.is_finalized():
        nc.finalize()
    return nc


def make_in_maps(x, y, W1, b1, W2, b2):
    x = np.asarray(x, dtype=np.float32)
    y = np.asarray(y, dtype=np.float32)
    W1 = np.asarray(W1, dtype=np.float32)
    b1 = np.asarray(b1, dtype=np.float32).reshape(HID)
    W2 = np.asarray(W2, dtype=np.float32).reshape(HID, 1)
    b2 = np.asarray(b2, dtype=np.float32).reshape(1)
    fdt = _np_mm_dtype()
    packed = MM_MODE == "bf16"

    w1p = np.ascontiguousarray(
        W1.reshape(4, 128, HID).transpose(1, 0, 2)).astype(fdt)
    if packed:
        b1r = np.ascontiguousarray(np.tile(b1, 4).reshape(128, 1))
        w2s = np.zeros((128, GS * (NG - 1) + HID), np.float32)
        for j in range(GS):
            w2s[32 * j:32 * j + 32, j + GS * (NG - 1)] = W2[:, 0]
    else:
        b1r = np.ascontiguousarray(b1.reshape(HID, 1))
        w2s = np.zeros((HID, 2 * NS - 1), np.float32)
        w2s[:, NS - 1] = W2[:, 0]
    w2s = w2s.astype(fdt)
    b2r = np.full((NS, 1), b2[0], np.float32)

    in_maps = []
    for c in range(NCORES):
        sl = slice(c * NS, (c + 1) * NS)
        # xh[i, p, j, a, t] = x[4i+j, t, a*128+p]
        xc = x[sl].reshape(NG, GS, T, 2, 128)
        xhc = np.ascontiguousarray(xc.transpose(0, 4, 1, 3, 2)).astype(fdt)
        yc = y[sl]
        ytp = np.ascontiguousarray(
            yc.T.reshape(2, 128, NS).transpose(1, 0, 2)).astype(fdt)
        in_maps.append({
            "xh": xhc, "ytp": ytp, "w1p": w1p, "b1r": b1r,
            "w2s": w2s, "b2r": b2r,
        })
    return in_maps


def run(x, y, W1, b1, W2, b2, trace=False, **run_kwargs):
    from concourse import bass_utils
    nc = build_bass()
    in_maps = make_in_maps(x, y, W1, b1, W2, b2)
    res = bass_utils.run_bass_kernel_spmd(
        nc, in_maps, core_ids=list(range(NCORES)), trace=trace, **run_kwargs,
    )
    shards = [np.asarray(res.results[c]["out"]) for c in range(NCORES)]
    full = np.concatenate(shards, axis=0).reshape(N, T, 1).astype(np.float32)
    return full, res


def kernel(x, y, x_length, W1, b1, W2, b2):
    full, _ = run(x, y, W1, b1, W2, b2, trace=False)
    return full


# revision 16
# speedup vs baseline: 2.4432x; 1.0549x over previous
"""Trainium2 Bass kernel for nn_Attention_57346403336437.

Math (per sample n):
    h1[t, :] = tanh(x[n,t,:] @ W1[:256] + y[n,:] @ W1[256:] + b1)      (T, 32)
    h2[t]    = relu(h1[t, :] @ W2 + b2)                                 (T,)
    a[t]     = exp(h2[t]) / (sum_t exp(h2[t]) + 1e-7)

Sharding: data-parallel over batch N=256 across 8 cores (32 samples each).
Weights replicated. Softmax-like normalization over T is core-local.

Device layout (per core):
  - x arrives host-pretiled as xh (NG, 128, GS, 2, T): partition dim =
    d%128, so every group load is one fully-linear dma_start (16KB
    contiguous per partition) -> near-peak HBM bandwidth.
  - bf16 mode (default): 4 samples packed per PSUM tile [128(4n x 32h), T];
    q matmuls col-tiled at tile_position (0, 32j); one full-width tanh per
    group; h2 via shifted block-diagonal W2 accumulated into one [32, T]
    PSUM tile at partition base 0.
  - f32r fallback: fp32r forbids col-offset PSUM writes, so samples are
    processed one [32h, T] PSUM tile at a time (all matmuls base 0).
  - epilogue: exp (with b2 bias), clamp-at-1 (== exp o relu), row-sum,
    reciprocal, scale, store.
"""

import numpy as np

N, T, D, HID = 256, 512, 256, 32
NCORES = 8
NS = N // NCORES          # samples per core
GS = 4                    # samples per group / DMA batch
NG = NS // GS             # groups per core
EPS = 1e-7

# "bf16": fast (half DMA traffic, 1 cyc/row matmuls), ~5e-3 scale-rel err
# "f32r": precise (~2e-4), full f32 DMA traffic, ~2 cyc/row matmuls
MM_MODE = "bf16"


def _np_mm_dtype():
    if MM_MODE == "bf16":
        import ml_dtypes
        return ml_dtypes.bfloat16
    return np.float32


def build_bass():
    import concourse.bacc as bacc
    import concourse.tile as tile
    from concourse import mybir

    f32 = mybir.dt.float32
    fmm = {"bf16": mybir.dt.bfloat16,
           "f32r": mybir.dt.float32r,
           "f32": f32}[MM_MODE]
    packed = MM_MODE == "bf16"
    nc = bacc.Bacc(enable_partition_id=False)

    xh = nc.declare_dram_parameter("xh", [NG, 128, GS, 2, T], fmm, isOutput=False)
    ytp = nc.declare_dram_parameter("ytp", [128, 2, NS], fmm, isOutput=False)
    w1p = nc.declare_dram_parameter("w1p", [128, 4, HID], fmm, isOutput=False)
    out = nc.declare_dram_parameter("out", [NS, T], f32, isOutput=True)
    if packed:
        b1r = nc.declare_dram_parameter("b1r", [128, 1], f32, isOutput=False)
        w2s = nc.declare_dram_parameter("w2s", [128, GS * (NG - 1) + HID], fmm, isOutput=False)
    else:
        b1r = nc.declare_dram_parameter("b1r", [HID, 1], f32, isOutput=False)
        w2s = nc.declare_dram_parameter("w2s", [HID, 2 * NS - 1], fmm, isOutput=False)
    b2r = nc.declare_dram_parameter("b2r", [NS, 1], f32, isOutput=False)

    with tile.TileContext(nc) as tc:
        with (
            tc.tile_pool(name="singles", bufs=1) as singles,
            tc.tile_pool(name="xpool", bufs=6) as xpool,
            tc.tile_pool(name="hpool", bufs=3) as hpool,
            tc.tile_pool(name="opool", bufs=1) as opool,
            tc.tile_pool(name="psq", bufs=3, space="PSUM") as psq,
            tc.tile_pool(name="psc", bufs=1, space="PSUM") as psc,
            tc.tile_pool(name="psh", bufs=1, space="PSUM") as psh,
        ):
            # ---- one-time constants ----
            w1sb = singles.tile([128, 4, HID], fmm)     # [p, a, h]; d = a*128+p
            nc.scalar.dma_start(out=w1sb, in_=w1p[:])
            ytsb = singles.tile([128, 2, NS], fmm)      # [p, a, n]
            nc.scalar.dma_start(out=ytsb, in_=ytp[:])
            b1sb = singles.tile(list(b1r.shape), f32)
            nc.scalar.dma_start(out=b1sb, in_=b1r[:])
            w2sb = singles.tile(list(w2s.shape), fmm)
            nc.scalar.dma_start(out=w2sb, in_=w2s[:])
            b2sb = singles.tile([NS, 1], f32)
            nc.scalar.dma_start(out=b2sb, in_=b2r[:])

            Tanh = mybir.ActivationFunctionType.Tanh
            h2ps = psh.tile([NS, T], f32)               # accumulated over samples

            if packed:
                # cb[(j,h), i] = (y[4i+j] @ W1y)[h] + b1[h]
                cps = psc.tile([128, NG], f32)
                for j in range(GS):
                    for a in range(2):
                        nc.tensor.matmul(
                            out=cps[32 * j:32 * j + 32, :],
                            lhsT=w1sb[:, 2 + a, :],
                            rhs=ytsb[:, a, :].rearrange("p (i g) -> p g i", g=GS)[:, j, :],
                            start=(a == 0), stop=(a == 1),
                            tile_position=(0, 32 * j),
                        )
                cb = singles.tile([128, NG], f32)
                nc.vector.tensor_scalar_add(out=cb, in0=cps, scalar1=b1sb)

                xgs = []
                for i in range(NG):
                    xg = xpool.tile([128, GS, 2, T], fmm)
                    nc.sync.dma_start(out=xg, in_=xh[i])
                    xgs.append(xg)
                for i in range(NG):
                    xg = xgs[i]
                    qps = psq.tile([128, T], f32)
                    for j in range(GS):
                        for a in range(2):
                            nc.tensor.matmul(
                                out=qps[32 * j:32 * j + 32, :],
                                lhsT=w1sb[:, a, :],
                                rhs=xg[:, j, a, :],
                                start=(a == 0), stop=(a == 1),
                                tile_position=(0, 32 * j),
                            )
                    h1 = hpool.tile([128, T], fmm)
                    nc.scalar.activation(out=h1, in_=qps, func=Tanh,
                                         bias=cb[:, i:i + 1], scale=1.0)
                    # shifted block-diag W2: rows 4i..4i+3 of h2ps get this
                    # group's scores, zero elsewhere; accumulate over groups
                    nc.tensor.matmul(
                        out=h2ps,
                        lhsT=w2sb[:, GS * (NG - 1 - i):GS * (NG - 1 - i) + HID],
                        rhs=h1,
                        start=(i == 0), stop=(i == NG - 1),
                    )
            else:
                # cbb[h, n] = (y[n] @ W1y)[h] + b1[h]
                cps = psc.tile([HID, NS], f32)
                for a in range(2):
                    nc.tensor.matmul(out=cps, lhsT=w1sb[:, 2 + a, :],
                                     rhs=ytsb[:, a, :],
                                     start=(a == 0), stop=(a == 1))
                cbb = singles.tile([HID, NS], f32)
                nc.vector.tensor_scalar_add(out=cbb, in0=cps, scalar1=b1sb)

                for i in range(NG):
                    xg = xpool.tile([128, GS, 2, T], fmm)
                    nc.sync.dma_start(out=xg, in_=xh[i])
                    for j in range(GS):
                        n = GS * i + j
                        qps = psq.tile([HID, T], f32)
                        for a in range(2):
                            nc.tensor.matmul(out=qps, lhsT=w1sb[:, a, :],
                                             rhs=xg[:, j, a, :],
                                             start=(a == 0), stop=(a == 1))
                        h1 = hpool.tile([HID, T], fmm if MM_MODE == "bf16" else f32)
                        nc.scalar.activation(out=h1, in_=qps, func=Tanh,
                                             bias=cbb[:, n:n + 1], scale=1.0)
                        nc.tensor.matmul(
                            out=h2ps,
                            lhsT=w2sb[:, NS - 1 - n:2 * NS - 1 - n],
                            rhs=h1,
                            start=(n == 0), stop=(n == NS - 1),
                        )

            # ---- epilogue: a = max(exp(h2 + b2), 1) / (sum_t + eps) ----
            e = opool.tile([NS, T], f32)
            nc.scalar.activation(out=e, in_=h2ps,
                                 func=mybir.ActivationFunctionType.Exp,
                                 bias=b2sb, scale=1.0)
            ec = opool.tile([NS, T], f32)
            sums = opool.tile([NS, 1], f32)
            nc.vector.tensor_scalar(
                out=ec, in0=e, scalar1=1.0, scalar2=None,
                op0=mybir.AluOpType.max, op1=mybir.AluOpType.add,
                accum_out=sums,
            )
            seps = opool.tile([NS, 1], f32)
            nc.vector.tensor_scalar_add(out=seps, in0=sums, scalar1=EPS)
            rec = opool.tile([NS, 1], f32)
            nc.vector.reciprocal(out=rec, in_=seps)
            aout = opool.tile([NS, T], f32)
            nc.vector.tensor_scalar_mul(out=aout, in0=ec, scalar1=rec)
            nc.sync.dma_start(out=out[:], in_=aout)

    if not nc.is_finalized():
        nc.finalize()
    return nc


def make_in_maps(x, y, W1, b1, W2, b2):
    x = np.asarray(x, dtype=np.float32)
    y = np.asarray(y, dtype=np.float32)
    W1 = np.asarray(W1, dtype=np.float32)
    b1 = np.asarray(b1, dtype=np.float32).reshape(HID)
    W2 = np.asarray(W2, dtype=np.float32).reshape(HID, 1)
    b2 = np.asarray(b2, dtype=np.float32).reshape(1)
    fdt = _np_mm_dtype()
    packed = MM_MODE == "bf16"

    w1p = np.ascontiguousarray(
        W1.reshape(4, 128, HID).transpose(1, 0, 2)).astype(fdt)
    if packed:
        b1r = np.ascontiguousarray(np.tile(b1, 4).reshape(128, 1))
        w2s = np.zeros((128, GS * (NG - 1) + HID), np.float32)
        for j in range(GS):
            w2s[32 * j:32 * j + 32, j + GS * (NG - 1)] = W2[:, 0]
    else:
        b1r = np.ascontiguousarray(b1.reshape(HID, 1))
        w2s = np.zeros((HID, 2 * NS - 1), np.float32)
        w2s[:, NS - 1] = W2[:, 0]
    w2s = w2s.astype(fdt)
    b2r = np.full((NS, 1), b2[0], np.float32)

    in_maps = []
    for c in range(NCORES):
        sl = slice(c * NS, (c + 1) * NS)
        # xh[i, p, j, a, t] = x[4i+j, t, a*128+p]
        xc = x[sl].reshape(NG, GS, T, 2, 128)
        xhc = np.ascontiguousarray(xc.transpose(0, 4, 1, 3, 2)).astype(fdt)
        yc = y[sl]
        ytp = np.ascontiguousarray(
            yc.T.reshape(2, 128, NS).transpose(1, 0, 2)).astype(fdt)
        in_maps.append({
            "xh": xhc, "ytp": ytp, "w1p": w1p, "b1r": b1r,
            "w2s": w2s, "b2r": b2r,
        })
    return in_maps


def run(x, y, W1, b1, W2, b2, trace=False, **run_kwargs):
    from concourse import bass_utils
    nc = build_bass()
    in_maps = make_in_maps(x, y, W1, b1, W2, b2)
    res = bass_utils.run_bass_kernel_spmd(
        nc, in_maps, core_ids=list(range(NCORES)), trace=trace, **run_kwargs,
    )
    shards = [np.asarray(res.results[c]["out"]) for c in range(NCORES)]
    full = np.concatenate(shards, axis=0).reshape(N, T, 1).astype(np.float32)
    return full, res


def kernel(x, y, x_length, W1, b1, W2, b2):
    full, _ = run(x, y, W1, b1, W2, b2, trace=False)
    return full
